# revision 1
# baseline (speedup 1.0000x reference)
"""Pairwise squared-Euclidean distance matrix kernel for Trainium2.

Computes D[b, i, j] = ||A[b,i] - B[b,j]||^2 for A, B of shape [16, 4096, 256]
fp32, returning [16, 4096, 4096] fp32.

Sharding: data-parallel over the batch dim -- 2 batches per NeuronCore over
8 cores (SPMD: same program, different batch slices).

Per-core algorithm (per batch):
  1. Load B tiles [128, 256] fp32 in groups of 4 (SWDGE so input loads
     round-robin fairly against output writes), compute rB = sum(B^2) on
     ScalarE (activation Square + accum_out), PE-transpose each tile into
     bf16 B^T chunk tiles [128(d), 2(k), 512(j)] in SBUF. Each group's rB
     slice round-trips through DRAM (scatter + partition-broadcast load,
     HWDGE) to produce rB broadcast across partitions.
  2. For each 128-row block of A: load (SWDGE, prefetched a group ahead),
     compute rA ([128,1] per-partition), PE-transpose, scale by -2 into
     bf16 (folds the -2 of the cross term). For each pair of 512-wide j
     tiles: 4 accumulating bf16 matmuls (k = 256 = 2x128) into a 2-bank
     PSUM tile, then one VectorE scalar_tensor_tensor:
       out = (psum + rA) + rB_bcast            (fp32)
     After 4 pairs, DMA the [128, 4096] fp32 row block to the output.

The first 4 rows of batch 0 are emitted j-outer, interleaved with the B
preprocess, so output DMAs start as soon as the first chunk pairs land.
Batch b+1's B preprocess is interleaved into batch b's main loop so the
PE/DMA pipelines never drain at batch boundaries.

The bf16 rounding only affects the cross term; |error| ~ 0.1 against
|D| ~ 512, i.e. ~2e-4 relative.
"""

from contextlib import ExitStack

import numpy as np

import concourse.mybir as mybir
import concourse.tile as tile
from concourse import bacc
from concourse.bass import ts
from concourse.masks import make_identity

F32 = mybir.dt.float32
BF16 = mybir.dt.bfloat16

N_CORES = 8
FULL_BATCH = 16
N = 4096
D = 256
P = 128
NT = 512  # output j-tile width (one PSUM bank of fp32)
LOADG = 4  # natural-layout tiles coalesced per input DMA


def build_nc(b_per_core=FULL_BATCH // N_CORES, n=N, d=D):
    n_itiles = n // P
    n_jtiles = n // NT
    n_ktiles = d // P
    t_per_j = NT // P  # B tiles per bt chunk

    nc = bacc.Bacc()
    a_ext = nc.declare_dram_parameter("A", [b_per_core, n, d], F32, isOutput=False)
    b_ext = nc.declare_dram_parameter("B", [b_per_core, n, d], F32, isOutput=False)
    d_ext = nc.declare_dram_parameter("D", [b_per_core, n, n], F32, isOutput=True)

    with tile.TileContext(nc) as tc, ExitStack() as ctx:
        const_pool = ctx.enter_context(tc.tile_pool(name="const", bufs=1))
        nat_pool = ctx.enter_context(tc.tile_pool(name="nat", bufs=3))
        sq_pool = ctx.enter_context(tc.tile_pool(name="sq", bufs=2))
        bt_pool = ctx.enter_context(tc.tile_pool(name="bt", bufs=2 * n_jtiles))
        at_pool = ctx.enter_context(tc.tile_pool(name="at", bufs=6))
        r_pool = ctx.enter_context(tc.tile_pool(name="r", bufs=2))
        rbg_pool = ctx.enter_context(tc.tile_pool(name="rbg", bufs=10))
        ra_pool = ctx.enter_context(tc.tile_pool(name="ra", bufs=8))
        out_pool = ctx.enter_context(tc.tile_pool(name="out", bufs=5))
        psum_mm = ctx.enter_context(tc.tile_pool(name="psum_mm", bufs=3, space="PSUM"))
        psum_tr = ctx.enter_context(tc.tile_pool(name="psum_tr", bufs=2, space="PSUM"))
        dram_pool = ctx.enter_context(tc.tile_pool(name="dram", bufs=2, space="DRAM"))

        ident = const_pool.tile([P, P], F32)
        make_identity(nc, ident)

        bt_chunks = {}  # (b, jt) -> tile [P, n_ktiles, NT] bf16
        rb_bcast_tiles = {}  # b -> [P, n] f32

        GW = LOADG * P  # j-width covered by one B group (= NT when LOADG=4)
        n_bgroups = n_itiles // LOADG
        n_agroups = n_itiles // LOADG
        n_jpairs = max(n_jtiles // 2, 1)
        jts_pp = n_jtiles // n_jpairs  # j tiles per psum pair (2, or 1 small)

        def emit_b_group(b, g):
            """Load + process one group of LOADG natural B tiles, including
            this group's slice of the rB broadcast (per-group round trip so
            the first epilogues don't wait on the whole panel)."""
            bn = nat_pool.tile([P, LOADG, d], F32, tag="bn")
            nc.gpsimd.dma_start(
                bn[:],
                b_ext[b, ts(g, LOADG * P), :].rearrange("(t p) d -> p t d", p=P),
            )
            if g == 0:
                rb_bcast_tiles[b] = r_pool.tile(
                    [P, n], F32, tag="rb_bcast", name="rb_bcast"
                )
            r_bg = rbg_pool.tile([P, LOADG], F32, tag="rbg", name="r_bg")
            for tt in range(LOADG):
                t = g * LOADG + tt
                jt, tj = divmod(t, t_per_j)
                if tj == 0:
                    bt_chunks[(b, jt)] = bt_pool.tile(
                        [P, n_ktiles, NT], BF16, tag="bt", name="bt_chunk"
                    )
                chunk = bt_chunks[(b, jt)]
                sq = sq_pool.tile([P, d], F32, tag="sq")
                nc.scalar.activation(
                    sq[:],
                    bn[:, tt],
                    mybir.ActivationFunctionType.Square,
                    accum_out=r_bg[:, tt : tt + 1],
                )
                for k in range(n_ktiles):
                    ps = psum_tr.tile([P, P], F32, tag="ps_tr")
                    nc.tensor.transpose(ps[:], bn[:, tt, ts(k, P)], ident)
                    nc.scalar.copy(chunk[:, k, ts(tj, P)], ps[:])
            # rB round trip for this group's j-slice (HWDGE only -- keeps
            # the gpsimd Q7 free for SWDGE input-load descriptor generation)
            rb_dram = dram_pool.tile([GW], F32, tag="rb_dram", name="rb_dram")
            nc.sync.dma_start(rb_dram[:].rearrange("(t p) -> p t", p=P), r_bg[:])
            nc.sync.dma_start(
                rb_bcast_tiles[b][:, ts(g, GW)], rb_dram[:].partition_broadcast(P)
            )

        def load_a_group(b, g):
            t = nat_pool.tile([P, LOADG, d], F32, tag="an", name="an_group")
            nc.gpsimd.dma_start(
                t[:],
                a_ext[b, ts(g, LOADG * P), :].rearrange("(t p) d -> p t d", p=P),
            )
            return t

        def emit_a_row_pre(an):
            """rA square + A^T transpose/cast for one row -> (r_a, at)."""
            r_a = ra_pool.tile([P, 1], F32, tag="rA", name="r_a")
            sqa = sq_pool.tile([P, d], F32, tag="sqa")
            nc.scalar.activation(
                sqa[:],
                an,
                mybir.ActivationFunctionType.Square,
                accum_out=r_a[:],
            )
            at_tile = at_pool.tile([P, n_ktiles, P], BF16, tag="at", name="at_tile")
            for k in range(n_ktiles):
                ps = psum_tr.tile([P, P], F32, tag="ps_tr")
                nc.tensor.transpose(ps[:], an[:, ts(k, P)], ident)
                # fold the -2 of "-2*a.b" into the bf16 cast of A^T
                nc.scalar.mul(at_tile[:, k, :], ps[:], -2.0)
            return r_a, at_tile

        def emit_mm_pair(b, jp, r_a, at_tile, out_row):
            """4 matmuls into a 2-bank PSUM tile + one stt epilogue."""
            mm_ps = psum_mm.tile([P, jts_pp * NT], F32, tag="mm_ps", name="mm_ps")
            for jj in range(jts_pp):
                jt = jp * jts_pp + jj
                chunk = bt_chunks[(b, jt)]
                for k in range(n_ktiles):
                    nc.tensor.matmul(
                        mm_ps[:, ts(jj, NT)],
                        lhsT=at_tile[:, k, :],
                        rhs=chunk[:, k, :],
                        start=(k == 0),
                        stop=(k == n_ktiles - 1),
                    )
            nc.vector.scalar_tensor_tensor(
                out=out_row[:, ts(jp, jts_pp * NT)],
                in0=mm_ps[:],
                scalar=r_a[:],
                in1=rb_bcast_tiles[b][:, ts(jp, jts_pp * NT)],
                op0=mybir.AluOpType.add,
                op1=mybir.AluOpType.add,
            )

        an_groups = {0: load_a_group(0, 0)}

        # --- batch-0 startup: first LOADG rows emitted j-outer, interleaved
        # with the B preprocess, so output DMAs start as soon as the first
        # chunk pairs land instead of after the whole panel.
        groups_per_pair = max((jts_pp * NT) // GW, 1)
        pre_rows = min(LOADG, n_itiles)
        pre = [emit_a_row_pre(an_groups[0][:, r]) for r in range(pre_rows)]
        if n_agroups > 1 or b_per_core > 1:
            gnext = 1 % n_agroups
            an_groups[gnext] = load_a_group(0 if n_agroups > 1 else 1, gnext)
        pre_outs = [
            out_pool.tile([P, n], F32, tag="out_row", name="out_row")
            for _ in range(pre_rows)
        ]
        for g in range(n_bgroups):
            emit_b_group(0, g)
            if (g + 1) % groups_per_pair == 0:
                jp = g // groups_per_pair
                if jp < n_jpairs:
                    for r in range(pre_rows):
                        emit_mm_pair(0, jp, pre[r][0], pre[r][1], pre_outs[r])
        for r in range(pre_rows):
            nc.sync.dma_start(d_ext[0, ts(r, P), :], pre_outs[r][:])

        # --- main loop
        b_emitted = {0: n_bgroups}  # batch -> number of B groups emitted
        for b in range(b_per_core):
            for g in range(b_emitted.get(b, 0), n_bgroups):
                emit_b_group(b, g)  # catch-up (only for tiny configs)
                b_emitted[b] = g + 1
            for it in range(pre_rows if b == 0 else 0, n_itiles):
                # spread next batch's B preprocess across early iterations
                if b + 1 < b_per_core:
                    it0 = it - (pre_rows if b == 0 else 0)
                    if it0 < n_bgroups:
                        emit_b_group(b + 1, it0)
                        b_emitted[b + 1] = it0 + 1

                g, ti = divmod(it, LOADG)
                if ti == 0:
                    # prefetch the next A group one group ahead
                    if g + 1 < n_agroups:
                        an_groups[g + 1] = load_a_group(b, g + 1)
                    elif b + 1 < b_per_core:
                        an_groups[0] = load_a_group(b + 1, 0)
                an = an_groups[g][:, ti]
                r_a, at_tile = emit_a_row_pre(an)
                out_row = out_pool.tile([P, n], F32, tag="out_row")
                for jp in range(n_jpairs):
                    emit_mm_pair(b, jp, r_a, at_tile, out_row)
                nc.sync.dma_start(d_ext[b, ts(it, P), :], out_row[:])

    nc.compile()
    return nc


_NC_CACHE = {}


def _get_nc(b_per_core, n, d):
    key = (b_per_core, n, d)
    if key not in _NC_CACHE:
        _NC_CACHE[key] = build_nc(b_per_core, n, d)
    return _NC_CACHE[key]


def run(A, B, trace=False, trace_kwargs=None):
    """Run on hardware across 8 cores; returns (D_full, BassKernelResults)."""
    from concourse.bass_utils import run_bass_kernel_spmd

    A = np.ascontiguousarray(np.asarray(A, dtype=np.float32))
    B = np.ascontiguousarray(np.asarray(B, dtype=np.float32))
    full_b = A.shape[0]
    assert full_b % N_CORES == 0
    bpc = full_b // N_CORES
    nc = _get_nc(bpc, A.shape[1], A.shape[2])

    in_maps = [
        {
            "A": A[c * bpc : (c + 1) * bpc],
            "B": B[c * bpc : (c + 1) * bpc],
        }
        for c in range(N_CORES)
    ]
    res = run_bass_kernel_spmd(
        nc,
        in_maps,
        list(range(N_CORES)),
        trace=trace,
        **(trace_kwargs or {}),
    )
    out = np.concatenate([r["D"] for r in res.results], axis=0)
    return out, res


def kernel(A, B):
    out, _ = run(A, B, trace=False)
    return out



# revision 7
# speedup vs baseline: 1.4910x; 1.4910x over previous
"""Pairwise squared-Euclidean distance matrix kernel for Trainium2.

Computes D[b, i, j] = ||A[b,i] - B[b,j]||^2 for A, B of shape [16, 4096, 256]
fp32, returning [16, 4096, 4096] fp32.

Sharding: data-parallel over the batch dim -- 2 batches per NeuronCore over
8 cores (SPMD: same program, different batch slices).

v2 design (vs the fp32-out / bf16-matmul baseline):
  * Output is written to DRAM as bf16 and upcast to fp32 on the host.
    Halves the dominant HBM write stream (134 MB -> 67 MB per core).
    Quantization error ~1 ULP of ~512-magnitude values ~ 2e-3 relative.
  * The cross term -2*A.B is computed in fp8e4 (e4m3) with
    perf_mode=DoubleRow: one matmul per 512-wide j-tile contracts the
    full K=256 ([128, 2, :] operand layout), ~1.5x PE throughput vs bf16.
    The -2 is folded into the fp8 cast of A^T.
  * rB is computed from bf16 squares of the PE-transposed B panel via an
    all-ones matmul (reduces over partitions), which lands rB already
    BROADCAST across partitions in PSUM -- no DRAM round-trip.
  * The epilogue (PSUM -> +rA +rB -> bf16 SBUF) is split across three
    engines so none of them bottlenecks:
      - DVE: scalar_tensor_tensor on paired j-tiles ([128,1024] psum):
            out = (psum + rA) + rB
      - ScalarE+GpSimd on the other j-tiles: activation(Identity,
            bias=rA) evacuates PSUM -> bf16 tmp; gpsimd.tensor_add
            adds the broadcast rB (SBUF-only; GpSimd has no PSUM port).
    The DVE/ACT split rotates per row (planB every 4th row gives DVE
    3 pairs) to balance measured engine rates.

Error budget: fp8e4 cross term ~1.1 rms, bf16 output quant ~1.0 rms,
bf16 rB ~0.3 rms on |D| ~ 512 scale: rel l2 ~ 3e-3.
"""

from contextlib import ExitStack

import numpy as np

import concourse.mybir as mybir
import concourse.tile as tile
from concourse import bacc
from concourse.bass import ts
from concourse.masks import make_identity

F32 = mybir.dt.float32
BF16 = mybir.dt.bfloat16
FP8 = mybir.dt.float8e4
AF = mybir.ActivationFunctionType
ALU = mybir.AluOpType

N_CORES = 8
FULL_BATCH = 16
N = 4096
D = 256
P = 128
NT = 512  # output j-tile width (one PSUM bank of fp32)
LOADG = 4  # natural-layout tiles coalesced per input DMA (= NT/P)
PLANB_EVERY = 4  # every PLANB_EVERY-th row uses the DVE-heavier plan


def make_plans(n_jtiles):
    """Per-row epilogue plans: list of ("dvep", jt0) | ("dves", jt) |
    ("act", jt). "dvep" covers jt0 and jt0+1 with one paired DVE stt."""
    if n_jtiles == 1:
        return [[("dves", 0)]]
    if n_jtiles == 2:
        return [[("dves", 0), ("act", 1)]]
    assert n_jtiles % 4 == 0
    plan_a = []
    for j0 in range(0, n_jtiles, 4):
        plan_a += [("dvep", j0), ("act", j0 + 2), ("act", j0 + 3)]
    plan_b = [("dvep", j) for j in range(0, n_jtiles - 2, 2)]
    plan_b += [("act", n_jtiles - 2), ("act", n_jtiles - 1)]
    return [plan_a, plan_b]


def build_nc(b_per_core=FULL_BATCH // N_CORES, n=N, d=D):
    n_itiles = n // P
    n_jtiles = n // NT
    n_ktiles = d // P
    t_per_j = NT // P  # B tiles per bt chunk
    assert n_ktiles == 2, "DoubleRow packing assumes K = 2*128"
    assert LOADG == t_per_j

    plans = make_plans(n_jtiles)

    def plan_for_row(it):
        if len(plans) > 1 and it % PLANB_EVERY == PLANB_EVERY - 1:
            return plans[1]
        return plans[0]

    nc = bacc.Bacc()
    a_ext = nc.declare_dram_parameter("A", [b_per_core, n, d], F32, isOutput=False)
    b_ext = nc.declare_dram_parameter("B", [b_per_core, n, d], F32, isOutput=False)
    d_ext = nc.declare_dram_parameter("D", [b_per_core, n, n], BF16, isOutput=True)

    with tile.TileContext(nc) as tc, ExitStack() as ctx:
        const_pool = ctx.enter_context(tc.tile_pool(name="const", bufs=1))
        nat_pool = ctx.enter_context(tc.tile_pool(name="nat", bufs=3))
        sqa_pool = ctx.enter_context(tc.tile_pool(name="sqa", bufs=2))
        sqb_pool = ctx.enter_context(tc.tile_pool(name="sqb", bufs=2))
        bt_pool = ctx.enter_context(tc.tile_pool(name="bt", bufs=2 * n_jtiles))
        rb_pool = ctx.enter_context(
            tc.tile_pool(name="rb", bufs=2 * max(n_jtiles // 2, 1))
        )
        at_pool = ctx.enter_context(tc.tile_pool(name="at", bufs=6))
        ra_pool = ctx.enter_context(tc.tile_pool(name="ra", bufs=8))
        tmp_pool = ctx.enter_context(tc.tile_pool(name="tmp", bufs=6))
        out_pool = ctx.enter_context(tc.tile_pool(name="out", bufs=6))
        # PSUM is 8 banks of [P, 512] fp32; pool bufs are bank-granular:
        # 2x2 (pairs) + 2x1 (singles) + 2x1 (transposes, packed 2/bank) = 8
        psum_pair = ctx.enter_context(tc.tile_pool(name="psum_pair", bufs=2, space="PSUM"))
        psum_sing = ctx.enter_context(tc.tile_pool(name="psum_sing", bufs=2, space="PSUM"))
        psum_tr = ctx.enter_context(tc.tile_pool(name="psum_tr", bufs=2, space="PSUM"))

        ident = const_pool.tile([P, P], F32)
        make_identity(nc, ident)
        # all-ones bf16 [P, P]: reduces over partitions in the rB matmul
        ones_t = const_pool.tile([P, P], BF16)
        nc.scalar.activation(ones_t[:], ident[:], AF.Identity, bias=1.0, scale=0.0)

        bt_chunks = {}  # (b, jt) -> [P, 2, NT] fp8 B^T chunk
        rb_pairs = {}  # (b, jp) -> [P, 2, NT] bf16 broadcast rB for jt 2jp, 2jp+1

        GW = LOADG * P  # j-width covered by one B group (== NT)
        n_bgroups = n_itiles // LOADG
        n_agroups = n_itiles // LOADG

        def emit_b_group(b, g):
            """Load + transpose one 512-wide B panel slice into an fp8
            chunk; square the (exact fp32) transposed tiles and reduce over
            partitions with an all-ones matmul to get broadcast rB."""
            bn = nat_pool.tile([P, LOADG, d], F32, tag="bn")
            nc.gpsimd.dma_start(
                bn[:],
                b_ext[b, ts(g, GW), :].rearrange("(t p) d -> p t d", p=P),
            )
            chunk = bt_pool.tile([P, n_ktiles, NT], FP8, tag="bt", name="bt_chunk")
            sqc = sqb_pool.tile([P, n_ktiles, NT], BF16, tag="sqb", name="sq_chunk")
            for tt in range(t_per_j):
                ps = psum_tr.tile([P, 2, P], F32, tag="ps_tr")
                for k in range(n_ktiles):
                    nc.tensor.transpose(ps[:, k, :], bn[:, tt, ts(k, P)], ident)
                    nc.scalar.copy(chunk[:, k, ts(tt, P)], ps[:, k, :])
                    nc.scalar.activation(sqc[:, k, ts(tt, P)], ps[:, k, :], AF.Square)
            jp, half = divmod(g, 2)
            if half == 0:
                rb_pairs[(b, jp)] = rb_pool.tile(
                    [P, 2, NT], BF16, tag="rb", name="rb_pair"
                )
            rb_ps = psum_sing.tile([P, NT], F32, tag="mm_sing", name="rb_ps")
            for k in range(n_ktiles):
                nc.tensor.matmul(
                    rb_ps[:],
                    lhsT=ones_t[:],
                    rhs=sqc[:, k, :],
                    start=(k == 0),
                    stop=(k == n_ktiles - 1),
                )
            nc.scalar.copy(rb_pairs[(b, jp)][:, half, :], rb_ps[:])
            bt_chunks[(b, g)] = chunk

        def load_a_group(b, g):
            t = nat_pool.tile([P, LOADG, d], F32, tag="an", name="an_group")
            nc.gpsimd.dma_start(
                t[:],
                a_ext[b, ts(g, LOADG * P), :].rearrange("(t p) d -> p t d", p=P),
            )
            return t

        def emit_a_row_pre(an):
            """rA square + A^T transpose and -2x fp8 cast for one row."""
            r_a = ra_pool.tile([P, 1], F32, tag="rA", name="r_a")
            sqa = sqa_pool.tile([P, d], F32, tag="sqa")
            nc.scalar.activation(sqa[:], an, AF.Square, accum_out=r_a[:])
            at_tile = at_pool.tile([P, n_ktiles, P], FP8, tag="at", name="at_tile")
            ps = psum_tr.tile([P, 2, P], F32, tag="ps_tr")
            for k in range(n_ktiles):
                nc.tensor.transpose(ps[:, k, :], an[:, ts(k, P)], ident)
                # fold the -2 of "-2*a.b" into the fp8 cast of A^T
                nc.scalar.mul(at_tile[:, k, :], ps[:, k, :], -2.0)
            return r_a, at_tile

        def mm_cross(out_ps, b, jt, at_tile):
            """One DoubleRow fp8 matmul: full K=256 cross term for a j-tile."""
            nc.tensor.matmul(
                out_ps,
                lhsT=at_tile[:, :, :],
                rhs=bt_chunks[(b, jt)][:, :, :],
                start=True,
                stop=True,
                perf_mode=mybir.MatmulPerfMode.DoubleRow,
            )

        def emit_item(b, item, r_a, at_tile, out_row):
            kind, jt = item
            if kind == "dvep":
                mm_ps = psum_pair.tile([P, 2 * NT], F32, tag="mm_pair", name="mm_pair")
                for jj in range(2):
                    mm_cross(mm_ps[:, ts(jj, NT)], b, jt + jj, at_tile)
                nc.vector.scalar_tensor_tensor(
                    out=out_row[:, jt * NT : (jt + 2) * NT],
                    in0=mm_ps[:],
                    scalar=r_a[:],
                    in1=rb_pairs[(b, jt // 2)][:, :, :],
                    op0=ALU.add,
                    op1=ALU.add,
                )
            elif kind == "dves":
                mm_ps = psum_sing.tile([P, NT], F32, tag="mm_sing", name="mm_sing")
                mm_cross(mm_ps[:], b, jt, at_tile)
                nc.vector.scalar_tensor_tensor(
                    out=out_row[:, ts(jt, NT)],
                    in0=mm_ps[:],
                    scalar=r_a[:],
                    in1=rb_pairs[(b, jt // 2)][:, jt % 2, :],
                    op0=ALU.add,
                    op1=ALU.add,
                )
            else:  # "act": ScalarE evacuates psum with +rA; GpSimd adds rB
                mm_ps = psum_sing.tile([P, NT], F32, tag="mm_sing", name="mm_sing")
                mm_cross(mm_ps[:], b, jt, at_tile)
                tmp = tmp_pool.tile([P, NT], BF16, tag="tmp", name="act_tmp")
                nc.scalar.activation(
                    tmp[:], mm_ps[:], AF.Identity, bias=r_a[:], scale=1.0
                )
                nc.gpsimd.tensor_add(
                    out_row[:, ts(jt, NT)],
                    tmp[:],
                    rb_pairs[(b, jt // 2)][:, jt % 2, :],
                )

        an_groups = {0: load_a_group(0, 0)}

        # --- batch-0 startup: first LOADG rows emitted j-outer, interleaved
        # with the B preprocess, so output DMAs start as soon as chunks land.
        pre_rows = min(LOADG, n_itiles)
        pre = [emit_a_row_pre(an_groups[0][:, r]) for r in range(pre_rows)]
        if n_agroups > 1 or b_per_core > 1:
            gnext = 1 % n_agroups
            an_groups[gnext] = load_a_group(0 if n_agroups > 1 else 1, gnext)
        pre_outs = [
            out_pool.tile([P, n], BF16, tag="out_row", name="out_row")
            for _ in range(pre_rows)
        ]
        warm_plan = plans[0]
        # item -> last B group it needs (chunk jt == group g)
        item_last_g = {
            item: (item[1] + 1 if item[0] == "dvep" else item[1])
            for item in warm_plan
        }
        for g in range(n_bgroups):
            emit_b_group(0, g)
            for item in warm_plan:
                if item_last_g[item] == g:
                    for r in range(pre_rows):
                        emit_item(0, item, pre[r][0], pre[r][1], pre_outs[r])
        for r in range(pre_rows):
            nc.sync.dma_start(d_ext[0, ts(r, P), :], pre_outs[r][:])

        # --- main loop
        b_emitted = {0: n_bgroups}  # batch -> number of B groups emitted
        for b in range(b_per_core):
            for g in range(b_emitted.get(b, 0), n_bgroups):
                emit_b_group(b, g)  # catch-up (only for tiny configs)
                b_emitted[b] = g + 1
            for it in range(pre_rows if b == 0 else 0, n_itiles):
                # spread next batch's B preprocess across early iterations
                if b + 1 < b_per_core:
                    it0 = it - (pre_rows if b == 0 else 0)
                    if it0 < n_bgroups:
                        emit_b_group(b + 1, it0)
                        b_emitted[b + 1] = it0 + 1

                g, ti = divmod(it, LOADG)
                if ti == 0:
                    # prefetch the next A group one group ahead
                    if g + 1 < n_agroups:
                        an_groups[g + 1] = load_a_group(b, g + 1)
                    elif b + 1 < b_per_core:
                        an_groups[0] = load_a_group(b + 1, 0)
                an = an_groups[g][:, ti]
                r_a, at_tile = emit_a_row_pre(an)
                out_row = out_pool.tile([P, n], BF16, tag="out_row")
                for item in plan_for_row(it):
                    emit_item(b, item, r_a, at_tile, out_row)
                nc.sync.dma_start(d_ext[b, ts(it, P), :], out_row[:])

    nc.compile()
    return nc


_NC_CACHE = {}


def _get_nc(b_per_core, n, d):
    key = (b_per_core, n, d)
    if key not in _NC_CACHE:
        _NC_CACHE[key] = build_nc(b_per_core, n, d)
    return _NC_CACHE[key]


def run(A, B, trace=False, trace_kwargs=None):
    """Run on hardware across 8 cores; returns (D_full, BassKernelResults)."""
    from concourse.bass_utils import run_bass_kernel_spmd

    A = np.ascontiguousarray(np.asarray(A, dtype=np.float32))
    B = np.ascontiguousarray(np.asarray(B, dtype=np.float32))
    full_b = A.shape[0]
    assert full_b % N_CORES == 0
    bpc = full_b // N_CORES
    nc = _get_nc(bpc, A.shape[1], A.shape[2])

    in_maps = [
        {
            "A": A[c * bpc : (c + 1) * bpc],
            "B": B[c * bpc : (c + 1) * bpc],
        }
        for c in range(N_CORES)
    ]
    res = run_bass_kernel_spmd(
        nc,
        in_maps,
        list(range(N_CORES)),
        trace=trace,
        **(trace_kwargs or {}),
    )
    out = np.concatenate(
        [np.asarray(r["D"]).astype(np.float32) for r in res.results], axis=0
    )
    return out, res


def kernel(A, B):
    out, _ = run(A, B, trace=False)
    return out


# revision 8
# speedup vs baseline: 1.8617x; 1.2487x over previous
"""Pairwise squared-Euclidean distance matrix kernel for Trainium2.

Computes D[b, i, j] = ||A[b,i] - B[b,j]||^2 for A, B of shape [16, 4096, 256]
fp32, returning [16, 4096, 4096] fp32.

Sharding: data-parallel over the batch dim -- 2 batches per NeuronCore over
8 cores (SPMD: same program, different batch slices).

Design (v3):
  * Output is written to DRAM as bf16 and upcast to fp32 on the host.
    Halves the dominant HBM write stream (134 MB -> 67 MB per core).
  * The cross term -2*A.B is computed in fp8e4 (e4m3) with
    perf_mode=DoubleRow: one matmul per 512-wide j-tile contracts the
    full K=256 ([128, 2, :] operand layout), ~1.5x PE throughput vs bf16.
    The -2 is folded into the fp8 cast of A^T.
  * rB is computed from bf16 squares of the (exact, fp32) PE-transposed B
    panel via an all-ones matmul (reduces over partitions), which lands rB
    already BROADCAST across partitions in PSUM -- no DRAM round-trip.
  * The epilogue (PSUM -> +rA +rB -> bf16 SBUF) works on [128, 1024]
    j-tile PAIRS and is split across three engines so none bottlenecks:
      - DVE pairs:   scalar_tensor_tensor  out = (psum + rA) + rB
      - ACT+GP pairs: activation(Identity, bias=rA) evacuates PSUM ->
            bf16 tmp on ScalarE; gpsimd.tensor_add adds the broadcast rB
            (SBUF-only; GpSimd has no PSUM port).
    The assignment rotates per row (~1.25 ACT pairs of 4) to balance
    measured engine rates (DVE ~0.64us, ACT ~0.51us, GP ~1.15us per
    512-wide j-tile).
  * All ScalarE preprocess ops run at [P, 2, 128] / row granularity
    (both k-chunks in one instruction) to amortize per-op overhead.

Error budget: fp8e4 cross term ~1.1 rms, bf16 output quant ~1.0 rms,
bf16 rB ~0.3 rms on |D| ~ 512 scale: rel l2 ~ 3e-3.
"""

from contextlib import ExitStack

import numpy as np

import concourse.mybir as mybir
import concourse.tile as tile
from concourse import bacc
from concourse.bass import ts
from concourse.masks import make_identity

F32 = mybir.dt.float32
BF16 = mybir.dt.bfloat16
FP8 = mybir.dt.float8e4
AF = mybir.ActivationFunctionType
ALU = mybir.AluOpType

N_CORES = 8
FULL_BATCH = 16
N = 4096
D = 256
P = 128
NT = 512  # output j-tile width (one PSUM bank of fp32)
LOADG = 4  # natural-layout tiles coalesced per input DMA (= NT/P)


def make_row_plans(n_jtiles):
    """Cycle of per-row epilogue plans; each plan is a list of
    ("dvep"|"actp", j0) pair items covering j-tiles j0, j0+1."""
    if n_jtiles == 1:
        return [[("dve1", 0)]]
    if n_jtiles == 2:
        return [[("dvep", 0)], [("actp", 0)]]
    assert n_jtiles % 2 == 0
    pair_starts = list(range(0, n_jtiles, 2))
    # rotate one ACT pair through the slots; every 4th row gets two
    act_cycle = [(0,), (4 % n_jtiles,), (2,), (6 % n_jtiles, 2)]
    plans = []
    for acts in act_cycle:
        plans.append(
            [("actp" if j0 in acts else "dvep", j0) for j0 in pair_starts]
        )
    return plans


def build_nc(b_per_core=FULL_BATCH // N_CORES, n=N, d=D):
    n_itiles = n // P
    n_jtiles = n // NT
    n_ktiles = d // P
    t_per_j = NT // P  # B tiles per bt chunk
    assert n_ktiles == 2, "DoubleRow packing assumes K = 2*128"
    assert LOADG == t_per_j

    plans = make_row_plans(n_jtiles)

    nc = bacc.Bacc()
    a_ext = nc.declare_dram_parameter("A", [b_per_core, n, d], F32, isOutput=False)
    b_ext = nc.declare_dram_parameter("B", [b_per_core, n, d], F32, isOutput=False)
    d_ext = nc.declare_dram_parameter("D", [b_per_core, n, n], BF16, isOutput=True)

    with tile.TileContext(nc) as tc, ExitStack() as ctx:
        const_pool = ctx.enter_context(tc.tile_pool(name="const", bufs=1))
        nat_pool = ctx.enter_context(tc.tile_pool(name="nat", bufs=3))
        sqa_pool = ctx.enter_context(tc.tile_pool(name="sqa", bufs=2))
        sqb_pool = ctx.enter_context(tc.tile_pool(name="sqb", bufs=2))
        bt_pool = ctx.enter_context(tc.tile_pool(name="bt", bufs=2 * n_jtiles))
        rb_pool = ctx.enter_context(
            tc.tile_pool(name="rb", bufs=2 * max(n_jtiles // 2, 1))
        )
        at_pool = ctx.enter_context(tc.tile_pool(name="at", bufs=6))
        ra_pool = ctx.enter_context(tc.tile_pool(name="ra", bufs=8))
        tmp_pool = ctx.enter_context(tc.tile_pool(name="tmp", bufs=4))
        out_pool = ctx.enter_context(tc.tile_pool(name="out", bufs=6))
        # PSUM: 8 banks of [P, 512] fp32. 3x2 (pairs, shared by matmul
        # accumulation and the rB build) + 2x1 (transposes, 2 per bank) = 8
        psum_pair = ctx.enter_context(tc.tile_pool(name="psum_pair", bufs=3, space="PSUM"))
        psum_tr = ctx.enter_context(tc.tile_pool(name="psum_tr", bufs=2, space="PSUM"))

        ident = const_pool.tile([P, P], F32)
        make_identity(nc, ident)
        # all-ones bf16 [P, P]: reduces over partitions in the rB matmul
        ones_t = const_pool.tile([P, P], BF16)
        nc.scalar.activation(ones_t[:], ident[:], AF.Identity, bias=1.0, scale=0.0)

        bt_chunks = {}  # (b, jt) -> [P, 2, NT] fp8 B^T chunk
        rb_pairs = {}  # (b, jp) -> [P, 2, NT] bf16 broadcast rB for jt 2jp, 2jp+1

        GW = LOADG * P  # j-width covered by one B group (== NT)
        n_bgroups = n_itiles // LOADG
        n_agroups = n_itiles // LOADG

        def emit_b_group(b, g):
            """Load + transpose one 512-wide B panel slice into an fp8
            chunk; square the (exact fp32) transposed tiles and reduce over
            partitions with an all-ones matmul to get broadcast rB."""
            bn = nat_pool.tile([P, LOADG, d], F32, tag="bn")
            nc.gpsimd.dma_start(
                bn[:],
                b_ext[b, ts(g, GW), :].rearrange("(t p) d -> p t d", p=P),
            )
            chunk = bt_pool.tile([P, n_ktiles, NT], FP8, tag="bt", name="bt_chunk")
            sqc = sqb_pool.tile([P, n_ktiles, NT], BF16, tag="sqb", name="sq_chunk")
            for tt in range(t_per_j):
                ps = psum_tr.tile([P, 2, P], F32, tag="ps_tr")
                for k in range(n_ktiles):
                    nc.tensor.transpose(ps[:, k, :], bn[:, tt, ts(k, P)], ident)
                # both k-chunks in one op: strided [P, 2, 128] out slices
                nc.scalar.copy(chunk[:, :, ts(tt, P)], ps[:])
                nc.scalar.activation(sqc[:, :, ts(tt, P)], ps[:], AF.Square)
            jp, half = divmod(g, 2)
            if half == 0:
                rb_pairs[(b, jp)] = rb_pool.tile(
                    [P, 2, NT], BF16, tag="rb", name="rb_pair"
                )
            rb_ps = psum_pair.tile([P, 2 * NT], F32, tag="mm_pair", name="rb_ps")
            for k in range(n_ktiles):
                nc.tensor.matmul(
                    rb_ps[:, :NT],
                    lhsT=ones_t[:],
                    rhs=sqc[:, k, :],
                    start=(k == 0),
                    stop=(k == n_ktiles - 1),
                )
            nc.scalar.copy(rb_pairs[(b, jp)][:, half, :], rb_ps[:, :NT])
            bt_chunks[(b, g)] = chunk

        def load_a_group(b, g):
            t = nat_pool.tile([P, LOADG, d], F32, tag="an", name="an_group")
            nc.gpsimd.dma_start(
                t[:],
                a_ext[b, ts(g, LOADG * P), :].rearrange("(t p) d -> p t d", p=P),
            )
            return t

        def emit_a_row_pre(an):
            """rA square + A^T transpose and -2x fp8 cast for one row."""
            r_a = ra_pool.tile([P, 1], F32, tag="rA", name="r_a")
            sqa = sqa_pool.tile([P, d], F32, tag="sqa")
            nc.scalar.activation(sqa[:], an, AF.Square, accum_out=r_a[:])
            at_tile = at_pool.tile([P, n_ktiles, P], FP8, tag="at", name="at_tile")
            ps = psum_tr.tile([P, 2, P], F32, tag="ps_tr")
            for k in range(n_ktiles):
                nc.tensor.transpose(ps[:, k, :], an[:, ts(k, P)], ident)
            # fold the -2 of "-2*a.b" into the fp8 cast of A^T (one op)
            nc.scalar.mul(at_tile[:, :, :], ps[:], -2.0)
            return r_a, at_tile

        def mm_cross(out_ps, b, jt, at_tile):
            """One DoubleRow fp8 matmul: full K=256 cross term for a j-tile."""
            nc.tensor.matmul(
                out_ps,
                lhsT=at_tile[:, :, :],
                rhs=bt_chunks[(b, jt)][:, :, :],
                start=True,
                stop=True,
                perf_mode=mybir.MatmulPerfMode.DoubleRow,
            )

        def emit_item(b, item, r_a, at_tile, out_row):
            kind, j0 = item
            mm_ps = psum_pair.tile([P, 2 * NT], F32, tag="mm_pair", name="mm_pair")
            if kind == "dve1":  # tiny configs: single j-tile via DVE
                mm_cross(mm_ps[:, :NT], b, j0, at_tile)
                nc.vector.scalar_tensor_tensor(
                    out=out_row[:, ts(j0, NT)],
                    in0=mm_ps[:, :NT],
                    scalar=r_a[:],
                    in1=rb_pairs[(b, j0 // 2)][:, j0 % 2, :],
                    op0=ALU.add,
                    op1=ALU.add,
                )
                return
            for jj in range(2):
                mm_cross(mm_ps[:, ts(jj, NT)], b, j0 + jj, at_tile)
            if kind == "dvep":
                nc.vector.scalar_tensor_tensor(
                    out=out_row[:, j0 * NT : (j0 + 2) * NT],
                    in0=mm_ps[:],
                    scalar=r_a[:],
                    in1=rb_pairs[(b, j0 // 2)][:, :, :],
                    op0=ALU.add,
                    op1=ALU.add,
                )
            else:  # "actp": ScalarE evacuates psum with +rA; GpSimd adds rB
                tmp = tmp_pool.tile([P, 2 * NT], BF16, tag="tmp", name="act_tmp")
                nc.scalar.activation(
                    tmp[:], mm_ps[:], AF.Identity, bias=r_a[:], scale=1.0
                )
                nc.gpsimd.tensor_add(
                    out_row[:, j0 * NT : (j0 + 2) * NT],
                    tmp[:],
                    rb_pairs[(b, j0 // 2)][:, :, :],
                )

        an_groups = {0: load_a_group(0, 0)}

        # --- batch-0 startup: first LOADG rows emitted j-outer, interleaved
        # with the B preprocess, so output DMAs start as soon as chunks land.
        pre_rows = min(LOADG, n_itiles)
        pre = [emit_a_row_pre(an_groups[0][:, r]) for r in range(pre_rows)]
        if n_agroups > 1 or b_per_core > 1:
            gnext = 1 % n_agroups
            an_groups[gnext] = load_a_group(0 if n_agroups > 1 else 1, gnext)
        pre_outs = [
            out_pool.tile([P, n], BF16, tag="out_row", name="out_row")
            for _ in range(pre_rows)
        ]
        # warmup rows r use plans[r % len(plans)]; emit each item as soon as
        # its last B chunk (group j0+1, or j0 for single) is processed
        for g in range(n_bgroups):
            emit_b_group(0, g)
            for r in range(pre_rows):
                for item in plans[r % len(plans)]:
                    last_g = item[1] + (1 if item[0] in ("dvep", "actp") else 0)
                    if last_g == g:
                        emit_item(0, item, pre[r][0], pre[r][1], pre_outs[r])
        for r in range(pre_rows):
            nc.sync.dma_start(d_ext[0, ts(r, P), :], pre_outs[r][:])

        # --- main loop
        b_emitted = {0: n_bgroups}  # batch -> number of B groups emitted
        for b in range(b_per_core):
            for g in range(b_emitted.get(b, 0), n_bgroups):
                emit_b_group(b, g)  # catch-up (only for tiny configs)
                b_emitted[b] = g + 1
            for it in range(pre_rows if b == 0 else 0, n_itiles):
                # spread next batch's B preprocess across early iterations
                if b + 1 < b_per_core:
                    it0 = it - (pre_rows if b == 0 else 0)
                    if it0 < n_bgroups:
                        emit_b_group(b + 1, it0)
                        b_emitted[b + 1] = it0 + 1

                g, ti = divmod(it, LOADG)
                if ti == 0:
                    # prefetch the next A group one group ahead
                    if g + 1 < n_agroups:
                        an_groups[g + 1] = load_a_group(b, g + 1)
                    elif b + 1 < b_per_core:
                        an_groups[0] = load_a_group(b + 1, 0)
                an = an_groups[g][:, ti]
                r_a, at_tile = emit_a_row_pre(an)
                out_row = out_pool.tile([P, n], BF16, tag="out_row")
                for item in plans[it % len(plans)]:
                    emit_item(b, item, r_a, at_tile, out_row)
                nc.sync.dma_start(d_ext[b, ts(it, P), :], out_row[:])

    nc.compile()
    return nc


_NC_CACHE = {}


def _get_nc(b_per_core, n, d):
    key = (b_per_core, n, d)
    if key not in _NC_CACHE:
        _NC_CACHE[key] = build_nc(b_per_core, n, d)
    return _NC_CACHE[key]


def run(A, B, trace=False, trace_kwargs=None):
    """Run on hardware across 8 cores; returns (D_full, BassKernelResults)."""
    from concourse.bass_utils import run_bass_kernel_spmd

    A = np.ascontiguousarray(np.asarray(A, dtype=np.float32))
    B = np.ascontiguousarray(np.asarray(B, dtype=np.float32))
    full_b = A.shape[0]
    assert full_b % N_CORES == 0
    bpc = full_b // N_CORES
    nc = _get_nc(bpc, A.shape[1], A.shape[2])

    in_maps = [
        {
            "A": A[c * bpc : (c + 1) * bpc],
            "B": B[c * bpc : (c + 1) * bpc],
        }
        for c in range(N_CORES)
    ]
    res = run_bass_kernel_spmd(
        nc,
        in_maps,
        list(range(N_CORES)),
        trace=trace,
        **(trace_kwargs or {}),
    )
    out = np.concatenate(
        [np.asarray(r["D"]).astype(np.float32) for r in res.results], axis=0
    )
    return out, res


def kernel(A, B):
    out, _ = run(A, B, trace=False)
    return out
